# revision 3
# baseline (speedup 1.0000x reference)
"""Trainium2 Bass kernel (v11 = v6 with the z-update reassociated: t1 = z + k1/8 computed at eval 1 where DVE is idle, so the eval-2 DVE cluster ahead of the spine z-prime op is 2 ops shorter per chain) for nn_DirectRecurrentODE (spline-driven RK4 ODE).

v6 = v4 x 2 interleaved chains (batch 64 -> 2x32 per core): one chain's
cross-engine hop latency hides under the other chain's engine work.
Position-major emission so each engine FIFO alternates chains.

v4: minimum-matmul spine. 8 matmuls/step (HW charges a stationary reload
per fp32 matmul, so count dominates):
- Composite stationaries S1=[W1z; W1x] and S13=[(1/3)W1z; W1x] (96x128):
  each eval's pre1 update is ONE fused matmul whose rhs stacks the k-space
  operand (rows 0..63) and the spline delta (rows 64..95).
- The rhs staging slots live inside the DMA stream ring itself: the chunk
  DMA fills rows 64..95 (X deltas), ACT/DVE write the k/u/z operand into
  rows 0..63 at runtime. No copies.
- Spine per eval: tanh_h(ACT) -> mm2(PE) -> tanh_k(ACT) [-> one AXPY(DVE)
  for evals 3,4,1'] -> fused mm(PE). The E1 AXPY is the mandatory z-update
  itself (z' = zp2 + k4/8), so it costs nothing extra.
"""
import sys
import numpy as np

for _p in ("/opt/trn_rl_repo",):
    if _p not in sys.path:
        sys.path.append(_p)

import concourse.bass as bass
import concourse.bacc as bacc
import concourse.tile as tile
from concourse import mybir
from concourse.bass_utils import run_bass_kernel_spmd
from concourse import dve_ops
from concourse.dve_spec import Spec, Src0, Src1, C0, Zero, eq, select, lower
from concourse.dve_uop import DveOpSpec

F32 = mybir.dt.float32
AFT = mybir.ActivationFunctionType

B, L, C_IN, C_HID, C_HH, C_OUT = 512, 512, 32, 64, 128, 10
N_CORES = 8
BC = B // N_CORES          # batch per core (64)
T_FULL = L - 1             # number of RK4 steps (511)
CHUNK = 16                 # steps per coeff-stream DMA chunk
KROWS = C_HID + C_IN       # 96: fused rhs rows
NCH = 2                    # interleaved chains per core
BCH = BC // NCH            # batch per chain (32)


def _register_dve_op(name, spec, subdim=False):
    for op in dve_ops.OPS:
        if op.name == name:
            return op
    opcode = max(dve_ops._SUB_OPCODE_FOR_NAME.values()) + 1
    assert opcode < 0x20
    shas = {}
    for ver in ("v3", "v4"):
        try:
            uops = lower(spec, ver=ver)
            shas[ver] = DveOpSpec(
                name=name, opcode=opcode, uops=uops,
                rd1_en=dve_ops.has_src1(spec),
            ).sha(ver)
        except Exception:
            pass
    op = dve_ops.DveOp(name, spec, subdim=subdim, uops_sha=shas)
    dve_ops.OPS.append(op)
    dve_ops._SUB_OPCODE_FOR_NAME[name] = opcode
    dve_ops.CUSTOM_DVE_SPECS[name] = spec
    return op


AXPY = _register_dve_op(
    "ANT_AXPY",
    Spec(body=Src0 + C0 * Src1,
         reference=lambda in0, in1, c0, c1, c2: in0 + c0 * in1),
)

MASKSEL = _register_dve_op(
    "ANT_MASKSEL",
    Spec(body=select(eq(Src1, C0), Src0, Zero),
         reference=lambda in0, in1, c0, c1, c2: np.where(in1 == c0, in0, 0.0)),
)


def _spline_tables(times, a, b, c, d):
    a = np.asarray(a, np.float64)
    b_ = np.asarray(b, np.float64)
    c_ = np.asarray(c, np.float64)
    d_ = np.asarray(d, np.float64)
    tail = (a[:, -1] + b_[:, -1] + 0.5 * c_[:, -1] + d_[:, -1] / 3.0)[:, None]
    A = np.concatenate([a, tail], axis=1)  # [B, L, C]
    X13 = a + b_ / 3.0 + c_ / 18.0 + d_ / 81.0
    X23 = a + (2.0 / 3.0) * b_ + (2.0 / 9.0) * c_ + (8.0 / 81.0) * d_
    return A, X13, X23


def build_program(T=T_FULL, b1_nonzero=False, t_decl=None, repeats=1):
    nc = bacc.Bacc()
    n_chunks = ((t_decl or T) + CHUNK - 1) // CHUNK
    t_pad = n_chunks * CHUNK
    REC = 4 * BC  # floats per step per partition in the stream

    cf_in = nc.declare_dram_parameter("cf", [C_IN, t_pad, 4, BC], F32, isOutput=False)
    a0_in = nc.declare_dram_parameter("a0", [C_IN, BC], F32, isOutput=False)
    s1_in = nc.declare_dram_parameter("s1", [KROWS, C_HH], F32, isOutput=False)
    s13_in = nc.declare_dram_parameter("s13", [KROWS, C_HH], F32, isOutput=False)
    w2_in = nc.declare_dram_parameter("w2", [C_HH, C_HID], F32, isOutput=False)
    winit_in = nc.declare_dram_parameter("winit", [C_IN, C_HID], F32, isOutput=False)
    wout_in = nc.declare_dram_parameter("wout", [C_HID, C_OUT], F32, isOutput=False)
    bvec_in = nc.declare_dram_parameter("bvec", [4, 128], F32, isOutput=False)
    fi_in = nc.declare_dram_parameter("fi", [C_HID, BC], F32, isOutput=False)
    out_ext = nc.declare_dram_parameter("out", [C_OUT, BC], F32, isOutput=True)

    import contextlib
    with tile.TileContext(nc) as tc, contextlib.ExitStack() as ctx:
        singles = ctx.enter_context(tc.tile_pool(name="singles", bufs=1))
        cf_pool = ctx.enter_context(tc.tile_pool(name="cf", bufs=3))
        hpool = ctx.enter_context(tc.tile_pool(name="hpool", bufs=4))
        kpool = ctx.enter_context(tc.tile_pool(name="kpool", bufs=2))
        p1pool = ctx.enter_context(tc.tile_pool(name="p1", bufs=2, space="PSUM"))
        p2pool = ctx.enter_context(tc.tile_pool(name="p2", bufs=2, space="PSUM"))

        # ---- weights / constants ----
        a0t = singles.tile([128, BC], F32)
        nc.sync.dma_start(out=a0t[0:C_IN, :], in_=a0_in[:, :])
        s1 = singles.tile([128, C_HH], F32, name="s1")
        nc.sync.dma_start(out=s1[0:KROWS, :], in_=s1_in[:, :])
        s13 = singles.tile([128, C_HH], F32, name="s13")
        nc.sync.dma_start(out=s13[0:KROWS, :], in_=s13_in[:, :])
        w2 = singles.tile([128, C_HID], F32)
        nc.sync.dma_start(out=w2[:, :], in_=w2_in[:, :])
        winit = singles.tile([128, C_HID], F32)
        nc.sync.dma_start(out=winit[0:C_IN, :], in_=winit_in[:, :])
        wout = singles.tile([128, C_OUT], F32)
        nc.sync.dma_start(out=wout[0:C_HID, :], in_=wout_in[:, :])
        bvec = singles.tile([128, 4], F32)
        for r in range(4):
            nc.sync.dma_start(out=bvec[:, r:r + 1],
                              in_=bvec_in[r:r + 1, :].rearrange("o p -> p o"))
        fi_rep = singles.tile([128, BC], F32)
        nc.sync.dma_start(out=fi_rep[0:C_HID, :], in_=fi_in[:, :])

        zT = singles.tile([128, BC], F32)
        nc.vector.memset(zT[0:C_HID, :], 0.0)

        # ---- stream ring: rows 64..95 = X deltas (DMA); rows 0..63 are the
        # runtime-written k/u/z slots for the fused matmuls ----
        live = {}

        def load_chunk(chk):
            cft = cf_pool.tile([128, CHUNK * REC], F32, name="cft", tag="cft")
            nc.sync.dma_start(
                out=cft[C_HID:KROWS, :]
                    .rearrange("c (t e b) -> c t e b", t=CHUNK, e=4),
                in_=cf_in[:, chk * CHUNK:(chk + 1) * CHUNK, :, :],
            )
            return cft

        def slot(t, e, c, p0=0, p1_=KROWS):
            cft = live[t // CHUNK]
            ss = ((t % CHUNK) * 4 + e) * BC + c * BCH
            return cft[p0:p1_, ss:ss + BCH]

        # ---- z0 init: write z0 straight into slot(0,0) rows 0..63 ----
        live[0] = load_chunk(0)
        if n_chunks > 1:
            live[1] = load_chunk(1)
        for c in range(NCH):
            cols = slice(c * BCH, (c + 1) * BCH)
            p0t = p1pool.tile([128, BCH], F32, name="p1z", tag=f"p1_{c}")
            nc.tensor.matmul(p0t[0:C_HID, :], winit[0:C_IN, :],
                             a0t[0:C_IN, cols],
                             start=True, stop=True, tile_position=(0, 0))
            nc.scalar.activation(slot(0, 0, c, 0, C_HID), p0t[0:C_HID, :],
                                 AFT.Identity, bias=bvec[0:C_HID, 2:3])
            g = hpool.tile([128, BCH], F32, name="g", tag=f"g_{c}")
            nc.vector._custom_dve(MASKSEL, out=g[0:C_HID, :],
                                  in0=slot(0, 0, c, 0, C_HID),
                                  in1=fi_rep[0:C_HID, cols], s0=0.0)
            nc.vector.tensor_add(zT[0:C_HID, cols], zT[0:C_HID, cols],
                                 g[0:C_HID, :])

        b1bias = bvec[0:C_HH, 0:1] if b1_nonzero else None

        # ---- main scan ----
        for _rep in range(repeats):
         for t in range(T):
            if t % CHUNK == 0:
                c = t // CHUNK
                if c not in live:
                    live[c] = load_chunk(c)
                nxt = c + 1 if c + 1 < n_chunks else (
                    0 if _rep + 1 < repeats else None)
                if nxt is not None and nxt not in live:
                    live[nxt] = load_chunk(nxt)
                for kk in list(live):
                    if kk not in (c, nxt):
                        live.pop(kk)

            p1 = [p1pool.tile([128, BCH], F32, name="p1t", tag=f"p1_{c}")
                  for c in range(NCH)]
            k2t = [kpool.tile([128, BCH], F32, name="k2", tag=f"k2_{c}")
                   for c in range(NCH)]
            k3t = [kpool.tile([128, BCH], F32, name="k3", tag=f"k3_{c}")
                   for c in range(NCH)]
            k4t = [kpool.tile([128, BCH], F32, name="k4", tag=f"k4_{c}")
                   for c in range(NCH)]
            u4t = [hpool.tile([128, BCH], F32, name="u4t", tag=f"u4t_{c}")
                   for c in range(NCH)]
            s2 = [hpool.tile([128, BCH], F32, name="s2", tag=f"s2_{c}")
                  for c in range(NCH)]
            t1 = [hpool.tile([128, BCH], F32, name="t1", tag=f"zp_{c}")
                  for c in range(NCH)]
            zp2 = [hpool.tile([128, BCH], F32, name="zp2", tag=f"zp2_{c}")
                   for c in range(NCH)]
            zcur = [slot(t, 0, c, 0, C_HID) for c in range(NCH)]

            for e in range(4):
                st = s13 if e == 1 else s1
                for c in range(NCH):
                    nc.tensor.matmul(p1[c][:, :], st[0:KROWS, :], slot(t, e, c),
                                     start=(e == 0), stop=(e == 3),
                                     tile_position=(0, 0))
                hcur = [None] * NCH
                for c in range(NCH):
                    h = hpool.tile([128, BCH], F32, name="h", tag=f"h_{c}")
                    if b1bias is not None:
                        nc.scalar.activation(h[:, :], p1[c][:, :], AFT.Tanh,
                                             bias=b1bias)
                    else:
                        nc.scalar.activation(h[:, :], p1[c][:, :], AFT.Tanh)
                    hcur[c] = h
                p2 = [None] * NCH
                for c in range(NCH):
                    p2[c] = p2pool.tile([128, BCH], F32, name="p2t",
                                        tag=f"p2_{c}")
                    nc.tensor.matmul(p2[c][0:C_HID, :], w2[:, :], hcur[c][:, :],
                                     start=True, stop=True,
                                     tile_position=(0, 0))
                if e == 0:
                    for c in range(NCH):
                        nc.scalar.activation(slot(t, 1, c, 0, C_HID),
                                             p2[c][0:C_HID, :],
                                             AFT.Tanh, bias=bvec[0:C_HID, 1:2])
                elif e == 1:
                    for c in range(NCH):
                        nc.scalar.activation(k2t[c][0:C_HID, :],
                                             p2[c][0:C_HID, :],
                                             AFT.Tanh, bias=bvec[0:C_HID, 1:2])
                    for c in range(NCH):
                        nc.vector._custom_dve(AXPY, out=slot(t, 2, c, 0, C_HID),
                                              in0=k2t[c][0:C_HID, :],
                                              in1=slot(t, 1, c, 0, C_HID),
                                              s0=-2.0 / 3.0)
                    for c in range(NCH):
                        nc.vector._custom_dve(AXPY, out=u4t[c][0:C_HID, :],
                                              in0=slot(t, 1, c, 0, C_HID),
                                              in1=k2t[c][0:C_HID, :], s0=-1.5)
                    for c in range(NCH):
                        # early half of the z-update: t1 = z + k1/8
                        nc.vector._custom_dve(AXPY, out=t1[c][0:C_HID, :],
                                              in0=zcur[c],
                                              in1=slot(t, 1, c, 0, C_HID),
                                              s0=0.125)
                elif e == 2:
                    for c in range(NCH):
                        nc.scalar.activation(k3t[c][0:C_HID, :],
                                             p2[c][0:C_HID, :],
                                             AFT.Tanh, bias=bvec[0:C_HID, 1:2])
                    for c in range(NCH):
                        nc.vector._custom_dve(AXPY, out=slot(t, 3, c, 0, C_HID),
                                              in0=k3t[c][0:C_HID, :],
                                              in1=u4t[c][0:C_HID, :],
                                              s0=4.0 / 3.0)
                    for c in range(NCH):
                        nc.vector.tensor_add(s2[c][0:C_HID, :],
                                             k2t[c][0:C_HID, :],
                                             k3t[c][0:C_HID, :])
                    for c in range(NCH):
                        nc.vector._custom_dve(AXPY, out=zp2[c][0:C_HID, :],
                                              in0=t1[c][0:C_HID, :],
                                              in1=s2[c][0:C_HID, :], s0=0.375)
                else:
                    for c in range(NCH):
                        nc.scalar.activation(k4t[c][0:C_HID, :],
                                             p2[c][0:C_HID, :],
                                             AFT.Tanh, bias=bvec[0:C_HID, 1:2])
                    last = (_rep == repeats - 1) and (t == T - 1)
                    zouts = []
                    for c in range(NCH):
                        if last:
                            zdst = hpool.tile([128, BCH], F32, name="zl",
                                              tag=f"zp_{c}")
                            zout = zdst[0:C_HID, :]
                        else:
                            tn = t + 1 if t + 1 < T else 0
                            zout = slot(tn, 0, c, 0, C_HID)
                        nc.vector._custom_dve(AXPY, out=zout,
                                              in0=zp2[c][0:C_HID, :],
                                              in1=k4t[c][0:C_HID, :], s0=0.125)
                        zouts.append(zout)
                    for c in range(NCH):
                        cols = slice(c * BCH, (c + 1) * BCH)
                        g = hpool.tile([128, BCH], F32, name="g", tag=f"g_{c}")
                        nc.vector._custom_dve(MASKSEL, out=g[0:C_HID, :],
                                              in0=zouts[c],
                                              in1=fi_rep[0:C_HID, cols],
                                              s0=float(t + 1))
                        nc.vector.tensor_add(zT[0:C_HID, cols],
                                             zT[0:C_HID, cols],
                                             g[0:C_HID, :])

        # ---- readout ----
        ot = singles.tile([128, BC], F32)
        for c in range(NCH):
            cols = slice(c * BCH, (c + 1) * BCH)
            po = p2pool.tile([128, BCH], F32, name="po", tag=f"p2_{c}")
            nc.tensor.matmul(po[0:C_OUT, :], wout[0:C_HID, :],
                             zT[0:C_HID, cols],
                             start=True, stop=True, tile_position=(0, 0))
            nc.scalar.activation(ot[0:C_OUT, cols], po[0:C_OUT, :],
                                 AFT.Identity, bias=bvec[0:C_OUT, 3:4])
        nc.sync.dma_start(out=out_ext[:, :], in_=ot[0:C_OUT, :])

    nc.compile()
    return nc


def prepare_inputs(times, coeff_a, coeff_b, coeff_two_c, coeff_three_d,
                   final_index, W_init, b_init, W1, b1, W2, b2, W_out, b_out,
                   T=T_FULL):
    fi = np.asarray(final_index).astype(np.int64)
    W1 = np.asarray(W1, np.float32)
    b1 = np.asarray(b1, np.float32)
    W2_ = np.asarray(W2, np.float32)
    b2_ = np.asarray(b2, np.float32)
    W_init_ = np.asarray(W_init, np.float32)
    b_init_ = np.asarray(b_init, np.float32)
    W_out_ = np.asarray(W_out, np.float32)
    b_out_ = np.asarray(b_out, np.float32)

    A, X13, X23 = _spline_tables(times, coeff_a, coeff_b, coeff_two_c,
                                 coeff_three_d)
    b1_nonzero = bool(np.any(b1 != 0))
    n_chunks = (T + CHUNK - 1) // CHUNK
    t_pad = n_chunks * CHUNK

    At = np.transpose(A, (1, 2, 0))      # [L, C, B] float64
    X13t = np.transpose(X13, (1, 2, 0))  # [L-1, C, B]
    X23t = np.transpose(X23, (1, 2, 0))
    Xd = np.zeros((t_pad, 4, C_IN, B), np.float64)
    Xd[:T, 0] = At[:T]
    Xd[:T, 1] = (X13t - At[:L - 1])[:T]
    Xd[:T, 2] = (X23t - X13t)[:T]
    Xd[:T, 3] = (At[1:] - X23t)[:T]
    cf_all = np.ascontiguousarray(np.transpose(Xd, (2, 0, 1, 3)),
                                  np.float32)  # [c,t,e,b]
    a0_all = np.ascontiguousarray(At[0], np.float32)  # [C_IN, B]

    W1z = W1[:C_HID].astype(np.float64)
    W1x_ = W1[C_HID:].astype(np.float64)
    s1_arr = np.ascontiguousarray(
        np.concatenate([W1z, W1x_], axis=0).astype(np.float32))
    s13_arr = np.ascontiguousarray(
        np.concatenate([W1z / 3.0, W1x_], axis=0).astype(np.float32))

    bvec = np.zeros((4, 128), np.float32)
    bvec[0, :C_HH] = b1
    bvec[1, :C_HID] = b2_
    bvec[2, :C_HID] = b_init_
    bvec[3, :C_OUT] = b_out_

    in_maps = []
    for core in range(N_CORES):
        cols = slice(core * BC, (core + 1) * BC)
        cf_core = np.ascontiguousarray(cf_all[:, :, :, cols])
        fi_core = np.ascontiguousarray(
            np.broadcast_to(fi[cols].astype(np.float32), (C_HID, BC)))
        in_maps.append({
            "cf": cf_core,
            "a0": np.ascontiguousarray(a0_all[:, cols]),
            "s1": s1_arr,
            "s13": s13_arr,
            "w2": W2_,
            "winit": W_init_,
            "wout": W_out_,
            "bvec": bvec,
            "fi": fi_core,
        })
    return in_maps, b1_nonzero


_PROGRAM_CACHE = {}


def run(inputs, T=T_FULL, trace=False):
    in_maps, b1_nonzero = prepare_inputs(T=T, **inputs)
    key = (T, b1_nonzero)
    if key not in _PROGRAM_CACHE:
        _PROGRAM_CACHE[key] = build_program(T=T, b1_nonzero=b1_nonzero)
    nc = _PROGRAM_CACHE[key]
    res = run_bass_kernel_spmd(nc, in_maps, core_ids=list(range(N_CORES)),
                               trace=trace)
    outs = [res.results[c]["out"] for c in range(N_CORES)]  # [10, BC] each
    full = np.concatenate([o.T for o in outs], axis=0).astype(np.float32)
    return full, res


def kernel(**inputs):
    out, _ = run(inputs)
    return out
